# revision 3
# baseline (speedup 1.0000x reference)
"""SupCon loss kernel for Trainium2 (8 NeuronCores, SPMD row-sharded).

Math (matches the reference):
  S = (E @ E^T) / T,  T = 0.1
  pos_term_i = mean_{j != i, lab_j = lab_i} S_ij
  lse_i      = logsumexp_{j != i} S_ij
  loss       = -sum_i (pos_term_i - lse_i) / N * T

Per-core plan (core c owns rows c*1024 .. c*1024+1023):
  - Each core receives a column-ROTATED bf16 copy of E^T (own rows first),
    so the diagonal always falls in n-chunk t//4 at offset (t%4)*128 for
    m-tile t -- the program is identical across cores (pure SPMD).
  - PE: S row-block in [128 x 512] psum chunks (4 k-chunks of 128).
  - DVE: tensor_tensor_reduce fuses psum->SBUF copy + diag(-1e30) mask add
    + running row-max per chunk.
  - ACT: one activation(Exp, scale=10, bias=-10*rowmax, accum_out) per
    m-tile fuses exp + row-sum; Ln gives the logsumexp tail.
  - pos term via a tiny E @ G matmul (G = per-class embedding sums), with
    host-prepped one-hot/count weights; self-dot correction from host.
  - Output: per-row val_i = pos_term_S_i - lse_S_i as [128, 8] f32.
Host sums the 8 partial outputs -> loss = -total / N * T.
"""

import os
import sys

import numpy as np

for _p in (
    "/root/.axon_site",
    "/root/.axon_site/_ro/trn_rl_repo",
    "/root/.axon_site/_ro/pypackages",
    "/opt/trn_rl_repo",
):
    if os.path.isdir(_p) and _p not in sys.path:
        sys.path.append(_p)

import ml_dtypes

N, D, NCLS, NCORES = 8192, 512, 16, 8
ROWS = N // NCORES        # 1024 rows per core
MT = ROWS // 128          # 8 m-tiles per core
TEMP = 0.1
SCALE = 1.0 / TEMP        # 10.0
KC = D // 128             # 4 k-chunks
SEG = 2048                # DMA/rhs segment width (4 n-chunks each)
NSEG = N // SEG           # 4
NTC = N // 512            # 16 n-chunks per m-tile
BIG_NEG = -1.0e30

_PROG: dict = {}


def _build_program():
    if "nc" in _PROG:
        return _PROG["nc"]

    import concourse.tile as tile
    from concourse import bacc, mybir

    dt = mybir.dt
    Alu = mybir.AluOpType
    Act = mybir.ActivationFunctionType
    f32, bf16 = dt.float32, dt.bfloat16

    nc = bacc.Bacc("TRN2", target_bir_lowering=False, debug=False)

    etg_d = nc.dram_tensor("etg", [KC, 128, N + NCLS], bf16, kind="ExternalInput").ap()
    posw_d = nc.dram_tensor("posw", [128, MT, NCLS], f32, kind="ExternalInput").ap()
    posb_d = nc.dram_tensor("posb", [128, MT], f32, kind="ExternalInput").ap()
    diagb_d = nc.dram_tensor("diagb", [128, 896], f32, kind="ExternalInput").ap()
    out_d = nc.dram_tensor("out_vals", [128, MT], f32, kind="ExternalOutput").ap()

    with tile.TileContext(nc) as tc:
        with (
            tc.tile_pool(name="consts", bufs=1) as consts,
            tc.tile_pool(name="ets", bufs=1) as ets,
            tc.tile_pool(name="sbig", bufs=2) as sbig,
            tc.tile_pool(name="dump", bufs=1) as dump,
            tc.tile_pool(name="small", bufs=2) as small,
            tc.tile_pool(name="acc", bufs=1) as accp,
            tc.tile_pool(name="psum", bufs=7, space="PSUM") as psum,
            tc.tile_pool(name="pspos", bufs=1, space="PSUM") as pspos,
        ):
            # ---- constants / inputs resident in SBUF ----
            diagb = consts.tile([128, 896], f32)
            nc.sync.dma_start(diagb[:], diagb_d[:])
            zeros = consts.tile([128, 512], f32)
            nc.gpsimd.memset(zeros[:], 0.0)
            posw = consts.tile([128, MT, NCLS], f32)
            nc.sync.dma_start(posw[:], posw_d[:])
            posb = consts.tile([128, MT], f32)
            nc.sync.dma_start(posb[:], posb_d[:])

            # E^T (rotated) in 4 k-chunks x 4 column segments + G class sums
            et = [[None] * NSEG for _ in range(KC)]
            gcls = [None] * KC
            for s in range(NSEG):
                for k in range(KC):
                    ektile = ets.tile([128, SEG], bf16, name=f"et_{k}_{s}")
                    nc.sync.dma_start(
                        ektile[:], etg_d[k, :, s * SEG : (s + 1) * SEG]
                    )
                    et[k][s] = ektile
            for k in range(KC):
                gtile = ets.tile([128, NCLS], bf16, name=f"g_{k}")
                nc.sync.dma_start(gtile[:], etg_d[k, :, N : N + NCLS])
                gcls[k] = gtile

            vals = accp.tile([128, MT], f32)

            # chunks handled by DVE (rest go to ScalarE) -- balance the two
            DVE_NTS = {2, 5, 7, 10, 12, 15}

            for t in range(MT):
                s_sb = sbig.tile([128, N], f32, tag="s_sb")

                diag_nt = t // 4                   # rotated diag chunk
                o = (t % 4) * 128                  # offset inside that chunk

                for q in range(NSEG):
                    pss = [
                        psum.tile([128, 512], f32, name="ps", tag="ps")
                        for _ in range(4)
                    ]
                    for k in range(KC):
                        lhsT = et[k][0][:, t * 128 : (t + 1) * 128]
                        for j in range(4):
                            nc.tensor.matmul(
                                pss[j][:],
                                lhsT,
                                et[k][q][:, j * 512 : (j + 1) * 512],
                                start=(k == 0),
                                stop=(k == KC - 1),
                            )
                    for j in range(4):
                        nt = q * 4 + j
                        dst = s_sb[:, nt * 512 : (nt + 1) * 512]
                        if nt == diag_nt:
                            # psum + diag(-1e30) mask -> SBUF (DVE)
                            nc.vector.scalar_tensor_tensor(
                                out=dst,
                                in0=pss[j][:],
                                scalar=1.0,
                                in1=diagb[:, 384 - o : 896 - o],
                                op0=Alu.mult,
                                op1=Alu.add,
                            )
                        elif nt in DVE_NTS:
                            nc.vector.tensor_scalar(
                                dst, pss[j][:], 1.0, None, op0=Alu.mult
                            )
                        else:
                            nc.scalar.copy(dst, pss[j][:])

                # positive-term matmul: C = E_local @ G  -> [128, 16]
                cps = pspos.tile([128, NCLS], f32, tag="cps")
                for k in range(KC):
                    nc.tensor.matmul(
                        cps[:],
                        et[k][0][:, t * 128 : (t + 1) * 128],
                        gcls[k][:],
                        start=(k == 0),
                        stop=(k == KC - 1),
                    )

                rowmax = small.tile([128, 1], f32, tag="rowmax")
                nc.vector.tensor_reduce(
                    rowmax[:], s_sb[:], axis=mybir.AxisListType.X, op=Alu.max
                )
                negb = small.tile([128, 1], f32, tag="negb")
                nc.vector.tensor_scalar(
                    negb[:], rowmax[:], -float(SCALE), None, op0=Alu.mult
                )

                sumexp = small.tile([128, 1], f32, tag="sumexp")
                expd = dump.tile([128, N], f32, tag="expd")
                nc.scalar.activation(
                    expd[:],
                    s_sb[:],
                    Act.Exp,
                    bias=negb[:],
                    scale=float(SCALE),
                    accum_out=sumexp[:],
                )
                lnse = small.tile([128, 1], f32, tag="lnse")
                nc.scalar.activation(lnse[:], sumexp[:], Act.Ln)

                pos16 = small.tile([128, NCLS], f32, tag="pos16")
                posacc = small.tile([128, 1], f32, tag="posacc")
                nc.vector.scalar_tensor_tensor(
                    out=pos16[:],
                    in0=cps[:],
                    scalar=1.0,
                    in1=posw[:, t, :],
                    op0=Alu.mult,
                    op1=Alu.mult,
                    accum_out=posacc[:],
                )

                # val = (posacc - posb_t) + (negb - lnse)
                v1 = small.tile([128, 1], f32, tag="v1")
                nc.vector.tensor_sub(v1[:], negb[:], lnse[:])
                v2 = small.tile([128, 1], f32, tag="v2")
                nc.vector.tensor_sub(v2[:], posacc[:], posb[:, t : t + 1])
                nc.vector.tensor_add(vals[:, t : t + 1], v1[:], v2[:])

            nc.sync.dma_start(out_d[:], vals[:])

    nc.compile()
    _PROG["nc"] = nc
    return nc


def _prep_inputs(embeddings: np.ndarray, labels: np.ndarray):
    E = np.asarray(embeddings, dtype=np.float32)
    lab = np.asarray(labels).astype(np.int64)
    assert E.shape == (N, D) and lab.shape == (N,)

    Ebf = E.astype(ml_dtypes.bfloat16)
    Ef = Ebf.astype(np.float64)

    # per-class embedding sums (from the same bf16-rounded E the device sees)
    G = np.zeros((D, NCLS), np.float64)
    for l in range(NCLS):
        G[:, l] = Ef[lab == l].sum(axis=0)
    Gbf = G.astype(ml_dtypes.bfloat16)

    ET = np.ascontiguousarray(Ebf.T)              # [D, N] bf16

    cnt = np.bincount(lab, minlength=NCLS).astype(np.float64)
    cnt_i = cnt[lab] - 1.0                        # positives per anchor
    selfdot = (Ef * Ef).sum(axis=1)               # ||e_i||^2 (bf16-rounded E)
    posb_full = (SCALE * selfdot / cnt_i).astype(np.float32)
    posw_full = np.zeros((N, NCLS), np.float32)
    posw_full[np.arange(N), lab] = (SCALE / cnt_i).astype(np.float32)

    diagb = np.zeros((128, 896), np.float32)
    diagb[np.arange(128), np.arange(128) + 384] = BIG_NEG

    in_maps = []
    for c in range(NCORES):
        rot = np.roll(ET, -c * ROWS, axis=1)      # own columns first
        etg = np.concatenate([rot, Gbf], axis=1)  # [D, N+16]
        etg = np.ascontiguousarray(etg.reshape(KC, 128, N + NCLS))
        sl = slice(c * ROWS, (c + 1) * ROWS)
        posb_c = np.ascontiguousarray(posb_full[sl].reshape(MT, 128).T)
        posw_c = np.ascontiguousarray(
            posw_full[sl].reshape(MT, 128, NCLS).transpose(1, 0, 2)
        )
        in_maps.append(
            {
                "etg": etg,
                "posw": posw_c,
                "posb": posb_c,
                "diagb": diagb,
            }
        )
    return in_maps


def run(embeddings, labels, trace=False, tmpdir=None):
    """Build+run on 8 cores; returns (loss_scalar, BassKernelResults)."""
    from concourse.bass_utils import run_bass_kernel_spmd

    nc = _build_program()
    in_maps = _prep_inputs(embeddings, labels)
    res = run_bass_kernel_spmd(
        nc, in_maps, list(range(NCORES)), trace=trace, tmpdir=tmpdir
    )
    total = 0.0
    for r in res.results:
        total += float(r["out_vals"].astype(np.float64).sum())
    loss = -total / N * TEMP
    return np.float32(loss), res


def kernel(**inputs) -> np.ndarray:
    loss, _ = run(inputs["embeddings"], inputs["labels"])
    return loss


# revision 9
# speedup vs baseline: 1.1566x; 1.1566x over previous
"""SupCon loss kernel for Trainium2 (8 NeuronCores, SPMD row-sharded).

Math (matches the reference):
  S = (E @ E^T) / T,  T = 0.1
  pos_term_i = mean_{j != i, lab_j = lab_i} S_ij
  lse_i      = logsumexp_{j != i} S_ij
  loss       = -sum_i (pos_term_i - lse_i) / N * T

Per-core plan (core c owns rows c*1024 .. c*1024+1023):
  - Each core receives a column-ROTATED bf16 copy of E^T (own rows first),
    so the diagonal always falls in n-chunk t//4 at offset (t%4)*128 for
    m-tile t -- the program is identical across cores (pure SPMD).
  - PE: S row-block in [128 x 512] psum chunks (4 k-chunks of 128).
  - DVE: tensor_tensor_reduce fuses psum->SBUF copy + diag(-1e30) mask add
    + running row-max per chunk.
  - ACT: one activation(Exp, scale=10, bias=-10*rowmax, accum_out) per
    m-tile fuses exp + row-sum; Ln gives the logsumexp tail.
  - pos term via a tiny E @ G matmul (G = per-class embedding sums), with
    host-prepped one-hot/count weights; self-dot correction from host.
  - Output: per-row val_i = pos_term_S_i - lse_S_i as [128, 8] f32.
Host sums the 8 partial outputs -> loss = -total / N * T.
"""

import os
import sys

import numpy as np

for _p in (
    "/root/.axon_site",
    "/root/.axon_site/_ro/trn_rl_repo",
    "/root/.axon_site/_ro/pypackages",
    "/opt/trn_rl_repo",
):
    if os.path.isdir(_p) and _p not in sys.path:
        sys.path.append(_p)

import ml_dtypes

N, D, NCLS, NCORES = 8192, 512, 16, 8
ROWS = N // NCORES        # 1024 rows per core
MT = ROWS // 128          # 8 m-tiles per core
TEMP = 0.1
SCALE = 1.0 / TEMP        # 10.0
KC = D // 128             # 4 k-chunks
SEG = 2048                # DMA/rhs segment width (4 n-chunks each)
NSEG = N // SEG           # 4
NTC = N // 512            # 16 n-chunks per m-tile
BIG_NEG = -1.0e30

_PROG: dict = {}


def _build_program():
    if "nc" in _PROG:
        return _PROG["nc"]

    import concourse.tile as tile
    from concourse import bacc, mybir

    dt = mybir.dt
    Alu = mybir.AluOpType
    Act = mybir.ActivationFunctionType
    f32, bf16 = dt.float32, dt.bfloat16

    nc = bacc.Bacc("TRN2", target_bir_lowering=False, debug=False)

    etg_d = nc.dram_tensor("etg", [KC, 128, N + NCLS], bf16, kind="ExternalInput").ap()
    posw_d = nc.dram_tensor("posw", [128, MT, NCLS], f32, kind="ExternalInput").ap()
    posb_d = nc.dram_tensor("posb", [128, MT], f32, kind="ExternalInput").ap()
    diagb_d = nc.dram_tensor("diagb", [128, 896], f32, kind="ExternalInput").ap()
    out_d = nc.dram_tensor("out_vals", [128, MT, 2], f32, kind="ExternalOutput").ap()

    with tile.TileContext(nc) as tc:
        with (
            tc.tile_pool(name="consts", bufs=1) as consts,
            tc.tile_pool(name="ets", bufs=1) as ets,
            tc.tile_pool(name="sbig", bufs=2) as sbig,
            tc.tile_pool(name="dump", bufs=1) as dump,
            tc.tile_pool(name="small", bufs=2) as small,
            tc.tile_pool(name="acc", bufs=1) as accp,
            tc.tile_pool(name="psum", bufs=7, space="PSUM") as psum,
            tc.tile_pool(name="pspos", bufs=1, space="PSUM") as pspos,
        ):
            # ---- constants / inputs resident in SBUF ----
            diagb = consts.tile([128, 896], f32)
            nc.sync.dma_start(diagb[:], diagb_d[:])
            posw = consts.tile([128, MT, NCLS], f32)
            nc.sync.dma_start(posw[:], posw_d[:])
            posb = consts.tile([128, MT], f32)
            nc.sync.dma_start(posb[:], posb_d[:])

            # E^T (rotated) in 4 k-chunks x 4 column segments + G class sums
            et = [[None] * NSEG for _ in range(KC)]
            gcls = [None] * KC
            for s in range(NSEG):
                for k in range(KC):
                    ektile = ets.tile([128, SEG], bf16, name=f"et_{k}_{s}")
                    nc.sync.dma_start(
                        ektile[:], etg_d[k, :, s * SEG : (s + 1) * SEG]
                    )
                    et[k][s] = ektile
            for k in range(KC):
                gtile = ets.tile([128, NCLS], bf16, name=f"g_{k}")
                nc.sync.dma_start(gtile[:], etg_d[k, :, N : N + NCLS])
                gcls[k] = gtile

            # out[:, t, 0] = posacc - posb - 10*rowmax ; out[:, t, 1] = sumexp
            vals = accp.tile([128, MT, 2], f32)

            # chunks copied by DVE (rest go to ScalarE) -- balance the two
            DVE_NTS = {4, 7, 10, 13}

            for t in range(MT):
                s_sb = sbig.tile([128, N], f32, tag="s_sb")
                cmax4 = small.tile([128, NSEG], f32, tag="cmax4")

                diag_nt = t // 4                   # rotated diag chunk
                o = (t % 4) * 128                  # offset inside that chunk

                for q in range(NSEG):
                    pss = [
                        psum.tile([128, 512], f32, name="ps", tag="ps")
                        for _ in range(4)
                    ]
                    for k in range(KC):
                        lhsT = et[k][0][:, t * 128 : (t + 1) * 128]
                        for j in range(4):
                            nc.tensor.matmul(
                                pss[j][:],
                                lhsT,
                                et[k][q][:, j * 512 : (j + 1) * 512],
                                start=(k == 0),
                                stop=(k == KC - 1),
                            )
                    for j in range(4):
                        nt = q * 4 + j
                        dst = s_sb[:, nt * 512 : (nt + 1) * 512]
                        if nt == diag_nt:
                            # psum + diag(-1e30) mask -> SBUF (DVE)
                            nc.vector.scalar_tensor_tensor(
                                out=dst,
                                in0=pss[j][:],
                                scalar=1.0,
                                in1=diagb[:, 384 - o : 896 - o],
                                op0=Alu.mult,
                                op1=Alu.add,
                            )
                        elif nt in DVE_NTS:
                            nc.vector.tensor_scalar(
                                dst, pss[j][:], 1.0, None, op0=Alu.mult
                            )
                        else:
                            nc.scalar.copy(dst, pss[j][:])
                    # quad-level running max (short op: keeps PSUM draining)
                    nc.vector.tensor_reduce(
                        cmax4[:, q : q + 1],
                        s_sb[:, q * SEG : (q + 1) * SEG],
                        axis=mybir.AxisListType.X,
                        op=Alu.max,
                    )

                # positive-term matmul: C = E_local @ G  -> [128, 16]
                cps = pspos.tile([128, NCLS], f32, tag="cps")
                for k in range(KC):
                    nc.tensor.matmul(
                        cps[:],
                        et[k][0][:, t * 128 : (t + 1) * 128],
                        gcls[k][:],
                        start=(k == 0),
                        stop=(k == KC - 1),
                    )

                rowmax = small.tile([128, 1], f32, tag="rowmax")
                nc.vector.tensor_reduce(
                    rowmax[:], cmax4[:], axis=mybir.AxisListType.X, op=Alu.max
                )
                negb = small.tile([128, 1], f32, tag="negb")
                nc.vector.tensor_scalar(
                    negb[:], rowmax[:], -float(SCALE), None, op0=Alu.mult
                )

                expd = dump.tile([128, N], f32, tag="expd")
                nc.scalar.activation(
                    expd[:],
                    s_sb[:],
                    Act.Exp,
                    bias=negb[:],
                    scale=float(SCALE),
                    accum_out=vals[:, t, 1:2],
                )

                pos16 = small.tile([128, NCLS], f32, tag="pos16")
                posacc = small.tile([128, 1], f32, tag="posacc")
                nc.vector.scalar_tensor_tensor(
                    out=pos16[:],
                    in0=cps[:],
                    scalar=1.0,
                    in1=posw[:, t, :],
                    op0=Alu.mult,
                    op1=Alu.mult,
                    accum_out=posacc[:],
                )

                # out0 = (posacc - posb_t) + negb   (host adds -log(sumexp))
                v1 = small.tile([128, 1], f32, tag="v1")
                nc.vector.tensor_sub(v1[:], posacc[:], posb[:, t : t + 1])
                nc.vector.tensor_add(vals[:, t, 0:1], v1[:], negb[:])

            nc.sync.dma_start(out_d[:], vals[:])

    nc.compile()
    _PROG["nc"] = nc
    return nc


def _prep_inputs(embeddings: np.ndarray, labels: np.ndarray):
    E = np.asarray(embeddings, dtype=np.float32)
    lab = np.asarray(labels).astype(np.int64)
    assert E.shape == (N, D) and lab.shape == (N,)

    Ebf = E.astype(ml_dtypes.bfloat16)
    Ef = Ebf.astype(np.float64)

    # per-class embedding sums (from the same bf16-rounded E the device sees)
    G = np.zeros((D, NCLS), np.float64)
    for l in range(NCLS):
        G[:, l] = Ef[lab == l].sum(axis=0)
    Gbf = G.astype(ml_dtypes.bfloat16)

    ET = np.ascontiguousarray(Ebf.T)              # [D, N] bf16

    cnt = np.bincount(lab, minlength=NCLS).astype(np.float64)
    cnt_i = cnt[lab] - 1.0                        # positives per anchor
    selfdot = (Ef * Ef).sum(axis=1)               # ||e_i||^2 (bf16-rounded E)
    posb_full = (SCALE * selfdot / cnt_i).astype(np.float32)
    posw_full = np.zeros((N, NCLS), np.float32)
    posw_full[np.arange(N), lab] = (SCALE / cnt_i).astype(np.float32)

    diagb = np.zeros((128, 896), np.float32)
    diagb[np.arange(128), np.arange(128) + 384] = BIG_NEG

    in_maps = []
    for c in range(NCORES):
        rot = np.roll(ET, -c * ROWS, axis=1)      # own columns first
        etg = np.concatenate([rot, Gbf], axis=1)  # [D, N+16]
        etg = np.ascontiguousarray(etg.reshape(KC, 128, N + NCLS))
        sl = slice(c * ROWS, (c + 1) * ROWS)
        posb_c = np.ascontiguousarray(posb_full[sl].reshape(MT, 128).T)
        posw_c = np.ascontiguousarray(
            posw_full[sl].reshape(MT, 128, NCLS).transpose(1, 0, 2)
        )
        in_maps.append(
            {
                "etg": etg,
                "posw": posw_c,
                "posb": posb_c,
                "diagb": diagb,
            }
        )
    return in_maps


def run(embeddings, labels, trace=False, tmpdir=None):
    """Build+run on 8 cores; returns (loss_scalar, BassKernelResults)."""
    from concourse.bass_utils import run_bass_kernel_spmd

    nc = _build_program()
    in_maps = _prep_inputs(embeddings, labels)
    res = run_bass_kernel_spmd(
        nc, in_maps, list(range(NCORES)), trace=trace, tmpdir=tmpdir
    )
    total = 0.0
    for r in res.results:
        ov = r["out_vals"].astype(np.float64)
        total += float((ov[:, :, 0] - np.log(ov[:, :, 1])).sum())
    loss = -total / N * TEMP
    return np.float32(loss), res


def kernel(**inputs) -> np.ndarray:
    loss, _ = run(inputs["embeddings"], inputs["labels"])
    return loss


# revision 13
# speedup vs baseline: 1.2305x; 1.0639x over previous
"""SupCon loss kernel for Trainium2 (8 NeuronCores, SPMD row-sharded).

Math (matches the reference):
  S = (E @ E^T) / T,  T = 0.1
  pos_term_i = mean_{j != i, lab_j = lab_i} S_ij
  lse_i      = logsumexp_{j != i} S_ij
  loss       = -sum_i (pos_term_i - lse_i) / N * T

Per-core plan (core c owns rows c*1024 .. c*1024+1023):
  - Each core receives a column-ROTATED bf16 copy of E^T (own rows first),
    so the diagonal always falls in n-chunk t//4 at offset (t%4)*128 for
    m-tile t -- the program is identical across cores (pure SPMD).
  - PE: S row-block in [128 x 512] psum chunks (4 k-chunks of 128).
  - DVE: tensor_tensor_reduce fuses psum->SBUF copy + diag(-1e30) mask add
    + running row-max per chunk.
  - ACT: one activation(Exp, scale=10, bias=-10*rowmax, accum_out) per
    m-tile fuses exp + row-sum; Ln gives the logsumexp tail.
  - pos term via a tiny E @ G matmul (G = per-class embedding sums), with
    host-prepped one-hot/count weights; self-dot correction from host.
  - Output: per-row val_i = pos_term_S_i - lse_S_i as [128, 8] f32.
Host sums the 8 partial outputs -> loss = -total / N * T.
"""

import os
import sys

import numpy as np

for _p in (
    "/root/.axon_site",
    "/root/.axon_site/_ro/trn_rl_repo",
    "/root/.axon_site/_ro/pypackages",
    "/opt/trn_rl_repo",
):
    if os.path.isdir(_p) and _p not in sys.path:
        sys.path.append(_p)

import ml_dtypes

N, D, NCLS, NCORES = 8192, 512, 16, 8
ROWS = N // NCORES        # 1024 rows per core
MT = ROWS // 128          # 8 m-tiles per core
TEMP = 0.1
SCALE = 1.0 / TEMP        # 10.0
KC = D // 128             # 4 k-chunks
SEG = 2048                # DMA/rhs segment width (4 n-chunks each)
NSEG = N // SEG           # 4
NTC = N // 512            # 16 n-chunks per m-tile
BIG_NEG = -1.0e30

_PROG: dict = {}


def _build_program():
    if "nc" in _PROG:
        return _PROG["nc"]

    import concourse.tile as tile
    from concourse import bacc, mybir

    dt = mybir.dt
    Alu = mybir.AluOpType
    Act = mybir.ActivationFunctionType
    f32, bf16 = dt.float32, dt.bfloat16

    nc = bacc.Bacc("TRN2", target_bir_lowering=False, debug=False)

    etg_d = nc.dram_tensor("etg", [KC, 128, N + NCLS], bf16, kind="ExternalInput").ap()
    posw_d = nc.dram_tensor("posw", [128, MT, NCLS], f32, kind="ExternalInput").ap()
    posb_d = nc.dram_tensor("posb", [128, MT], f32, kind="ExternalInput").ap()
    diagb_d = nc.dram_tensor("diagb", [128, 896], f32, kind="ExternalInput").ap()
    out_d = nc.dram_tensor("out_vals", [128, MT, 2], f32, kind="ExternalOutput").ap()

    with tile.TileContext(nc) as tc:
        with (
            tc.tile_pool(name="consts", bufs=1) as consts,
            tc.tile_pool(name="ets", bufs=1) as ets,
            tc.tile_pool(name="sbig", bufs=2) as sbig,
            tc.tile_pool(name="dump", bufs=1) as dump,
            tc.tile_pool(name="small", bufs=2) as small,
            tc.tile_pool(name="acc", bufs=1) as accp,
            tc.tile_pool(name="psum", bufs=7, space="PSUM") as psum,
            tc.tile_pool(name="pspos", bufs=1, space="PSUM") as pspos,
        ):
            # ---- constants / inputs resident in SBUF ----
            diagb = consts.tile([128, 896], f32)
            nc.sync.dma_start(diagb[:], diagb_d[:])
            posw = consts.tile([128, MT, NCLS], f32)
            nc.sync.dma_start(posw[:], posw_d[:])
            posb = consts.tile([128, MT], f32)
            nc.sync.dma_start(posb[:], posb_d[:])

            # E^T (rotated) in 4 k-chunks x 4 column segments + G class sums
            et = [[None] * NSEG for _ in range(KC)]
            gcls = [None] * KC
            di = 0
            for s in range(NSEG):
                for k in range(KC):
                    ektile = ets.tile([128, SEG], bf16, name=f"et_{k}_{s}")
                    # alternate issuing engine: descriptor gen is ~0.7us each
                    eng = nc.sync if di % 2 == 0 else nc.gpsimd
                    eng.dma_start(ektile[:], etg_d[k, :, s * SEG : (s + 1) * SEG])
                    di += 1
                    et[k][s] = ektile
            for k in range(KC):
                gtile = ets.tile([128, NCLS], bf16, name=f"g_{k}")
                nc.gpsimd.dma_start(gtile[:], etg_d[k, :, N : N + NCLS])
                gcls[k] = gtile

            # out[:, t, 0] = posacc - posb - 10*rowmax ; out[:, t, 1] = sumexp
            vals = accp.tile([128, MT, 2], f32)

            # chunks copied by DVE (rest go to ScalarE) -- balance the two
            DVE_NTS = {3, 6, 9, 12, 15}

            for t in range(MT):
                s_sb = sbig.tile([128, N], f32, tag="s_sb")
                cmax4 = small.tile([128, NSEG], f32, tag="cmax4")
                negq4 = small.tile([128, NSEG], f32, tag="negq4")
                seq4 = small.tile([128, NSEG], f32, tag="seq4")
                expd = dump.tile([128, N], f32, tag="expd")

                diag_nt = t // 4                   # rotated diag chunk
                o = (t % 4) * 128                  # offset inside that chunk

                for q in range(NSEG):
                    pss = [
                        psum.tile([128, 512], f32, name="ps", tag="ps")
                        for _ in range(4)
                    ]
                    for k in range(KC):
                        lhsT = et[k][0][:, t * 128 : (t + 1) * 128]
                        for j in range(4):
                            nc.tensor.matmul(
                                pss[j][:],
                                lhsT,
                                et[k][q][:, j * 512 : (j + 1) * 512],
                                start=(k == 0),
                                stop=(k == KC - 1),
                            )
                    for j in range(4):
                        nt = q * 4 + j
                        dst = s_sb[:, nt * 512 : (nt + 1) * 512]
                        if nt == diag_nt:
                            # psum + diag(-1e30) mask -> SBUF (DVE)
                            nc.vector.scalar_tensor_tensor(
                                out=dst,
                                in0=pss[j][:],
                                scalar=1.0,
                                in1=diagb[:, 384 - o : 896 - o],
                                op0=Alu.mult,
                                op1=Alu.add,
                            )
                        elif nt in DVE_NTS:
                            nc.vector.tensor_scalar(
                                dst, pss[j][:], 1.0, None, op0=Alu.mult
                            )
                        else:
                            nc.scalar.copy(dst, pss[j][:])
                    # quad-level max + online exp (keeps both engines streaming)
                    nc.vector.tensor_reduce(
                        cmax4[:, q : q + 1],
                        s_sb[:, q * SEG : (q + 1) * SEG],
                        axis=mybir.AxisListType.X,
                        op=Alu.max,
                    )
                    nc.vector.tensor_scalar(
                        negq4[:, q : q + 1],
                        cmax4[:, q : q + 1],
                        -float(SCALE),
                        None,
                        op0=Alu.mult,
                    )
                    nc.scalar.activation(
                        expd[:, q * SEG : (q + 1) * SEG],
                        s_sb[:, q * SEG : (q + 1) * SEG],
                        Act.Exp,
                        bias=negq4[:, q : q + 1],
                        scale=float(SCALE),
                        accum_out=seq4[:, q : q + 1],
                    )

                # positive-term matmul: C = E_local @ G  -> [128, 16]
                cps = pspos.tile([128, NCLS], f32, tag="cps")
                for k in range(KC):
                    nc.tensor.matmul(
                        cps[:],
                        et[k][0][:, t * 128 : (t + 1) * 128],
                        gcls[k][:],
                        start=(k == 0),
                        stop=(k == KC - 1),
                    )

                # combine quads: sumexp = sum_q seq_q * exp(10*(cmax_q - rowmax))
                rowmax = small.tile([128, 1], f32, tag="rowmax")
                nc.vector.tensor_reduce(
                    rowmax[:], cmax4[:], axis=mybir.AxisListType.X, op=Alu.max
                )
                negb = small.tile([128, 1], f32, tag="negb")
                nc.vector.tensor_scalar(
                    negb[:], rowmax[:], -float(SCALE), None, op0=Alu.mult
                )
                e4 = small.tile([128, NSEG], f32, tag="e4")
                nc.scalar.activation(
                    e4[:], cmax4[:], Act.Exp, bias=negb[:], scale=float(SCALE)
                )
                t4 = small.tile([128, NSEG], f32, tag="t4")
                nc.vector.scalar_tensor_tensor(
                    out=t4[:],
                    in0=seq4[:],
                    scalar=1.0,
                    in1=e4[:],
                    op0=Alu.mult,
                    op1=Alu.mult,
                    accum_out=vals[:, t, 1:2],
                )

                pos16 = small.tile([128, NCLS], f32, tag="pos16")
                posacc = small.tile([128, 1], f32, tag="posacc")
                nc.vector.scalar_tensor_tensor(
                    out=pos16[:],
                    in0=cps[:],
                    scalar=1.0,
                    in1=posw[:, t, :],
                    op0=Alu.mult,
                    op1=Alu.mult,
                    accum_out=posacc[:],
                )

                # out0 = (posacc - posb_t) + negb   (host adds -log(sumexp))
                v1 = small.tile([128, 1], f32, tag="v1")
                nc.vector.tensor_sub(v1[:], posacc[:], posb[:, t : t + 1])
                nc.vector.tensor_add(vals[:, t, 0:1], v1[:], negb[:])

            nc.sync.dma_start(out_d[:], vals[:])

    nc.compile()
    _PROG["nc"] = nc
    return nc


def _prep_inputs(embeddings: np.ndarray, labels: np.ndarray):
    E = np.asarray(embeddings, dtype=np.float32)
    lab = np.asarray(labels).astype(np.int64)
    assert E.shape == (N, D) and lab.shape == (N,)

    Ebf = E.astype(ml_dtypes.bfloat16)
    Ef = Ebf.astype(np.float64)

    # per-class embedding sums (from the same bf16-rounded E the device sees)
    G = np.zeros((D, NCLS), np.float64)
    for l in range(NCLS):
        G[:, l] = Ef[lab == l].sum(axis=0)
    Gbf = G.astype(ml_dtypes.bfloat16)

    ET = np.ascontiguousarray(Ebf.T)              # [D, N] bf16

    cnt = np.bincount(lab, minlength=NCLS).astype(np.float64)
    cnt_i = cnt[lab] - 1.0                        # positives per anchor
    selfdot = (Ef * Ef).sum(axis=1)               # ||e_i||^2 (bf16-rounded E)
    posb_full = (SCALE * selfdot / cnt_i).astype(np.float32)
    posw_full = np.zeros((N, NCLS), np.float32)
    posw_full[np.arange(N), lab] = (SCALE / cnt_i).astype(np.float32)

    diagb = np.zeros((128, 896), np.float32)
    diagb[np.arange(128), np.arange(128) + 384] = BIG_NEG

    in_maps = []
    for c in range(NCORES):
        rot = np.roll(ET, -c * ROWS, axis=1)      # own columns first
        etg = np.concatenate([rot, Gbf], axis=1)  # [D, N+16]
        etg = np.ascontiguousarray(etg.reshape(KC, 128, N + NCLS))
        sl = slice(c * ROWS, (c + 1) * ROWS)
        posb_c = np.ascontiguousarray(posb_full[sl].reshape(MT, 128).T)
        posw_c = np.ascontiguousarray(
            posw_full[sl].reshape(MT, 128, NCLS).transpose(1, 0, 2)
        )
        in_maps.append(
            {
                "etg": etg,
                "posw": posw_c,
                "posb": posb_c,
                "diagb": diagb,
            }
        )
    return in_maps


def run(embeddings, labels, trace=False, tmpdir=None):
    """Build+run on 8 cores; returns (loss_scalar, BassKernelResults)."""
    from concourse.bass_utils import run_bass_kernel_spmd

    nc = _build_program()
    in_maps = _prep_inputs(embeddings, labels)
    res = run_bass_kernel_spmd(
        nc, in_maps, list(range(NCORES)), trace=trace, tmpdir=tmpdir
    )
    total = 0.0
    for r in res.results:
        ov = r["out_vals"].astype(np.float64)
        total += float((ov[:, :, 0] - np.log(ov[:, :, 1])).sum())
    loss = -total / N * TEMP
    return np.float32(loss), res


def kernel(**inputs) -> np.ndarray:
    loss, _ = run(inputs["embeddings"], inputs["labels"])
    return loss
